# revision 4
# baseline (speedup 1.0000x reference)
"""Trainium2 Bass kernel for nn_AttLstm (encoder LSTM + additive attention + decoder LSTM).

8-core tensor-parallel: core c owns H-slice [128c, 128c+128).
- Encoder/decoder LSTM gates sharded by H-slice (512 gates/core), orientation-A
  bf16 matvecs (weights stationary), gates/h in T-layout ([128, *] columns).
- Hidden state exchanged per step via ncfw AllGather (free-major) + PE transposes.
- Attention H-sharded; logit partials AllGather-summed; softmax without max-subtract
  (logits bounded by ||v2s||_1); z replicated from on-chip e_out history.

Self-contained: hardcodes S=512, I=H=1024, 8 cores.
"""
import os
import sys
import types

import numpy as np

# ---- NTFF profiling hook (optional, used when BASS_TRACE=1) ----
if "antenv.axon_hooks" not in sys.modules:
    _hooks_mod = types.ModuleType("antenv.axon_hooks")
    _HOOK = [None]
    _hooks_mod.set_axon_ntff_profile_hook = lambda h: _HOOK.__setitem__(0, h)
    _hooks_mod.get_axon_ntff_profile_hook = lambda: _HOOK[0]
    sys.modules["antenv.axon_hooks"] = _hooks_mod
    try:
        import antenv

        antenv.axon_hooks = _hooks_mod
    except ImportError:
        pass

import concourse.bass as bass
import concourse.bacc as bacc
import concourse.tile as tile
import concourse.mybir as mybir
import concourse.bass_utils as bass_utils

bass_utils.upload_artifacts = lambda tmpdir: tmpdir

N_CORES = 8
H = 1024
I_DIM = 1024
FP = mybir.dt.float32
BF = mybir.dt.bfloat16
AF = mybir.ActivationFunctionType
OP = mybir.AluOpType
NP_BF16 = mybir.dt.np(BF)

GMAP = [0, 1, 3, 2]  # local gate order [i, f, o, g] -> torch rows [i, f, g, o]

LAST_EXEC_NS = None


def build_kernel(S):
    """Build the full 8-core SPMD kernel for sequence length S (S % 128 == 0)."""
    st = S // 128  # s-tiles
    nc = bacc.Bacc("TRN2", target_bir_lowering=False, debug=False, num_devices=N_CORES)

    def din(name, shape, dt=BF):
        return nc.dram_tensor(name, shape, dt, kind="ExternalInput").ap()

    # weights, T-layout lhsT tiles (see host prep for exact element maps)
    whh = din("whh", [128, 8, 4, 128])
    wih = din("wih", [128, 8, 4, 128])
    xT = din("xT", [128, 8, S])
    ebrow = din("ebrow", [1, 4, 128], FP)  # encoder bias rows (per gate tile)
    atteh = din("atteh", [128, 8, 128])
    attch = din("attch", [128, 8, 128])
    bc = din("bc", [128, 1], FP)  # att_eh_b + att_bias + att_ch_b slice
    v2s = din("v2s", [128, 1])
    dw = din("dw", [128, 16, 4, 128])
    dbrow = din("dbrow", [128, 4], FP)  # decoder gate biases (columns)
    eye_f = din("eye_f", [128, 128], FP)
    eye_b = din("eye_b", [128, 128])
    ones8 = din("ones8", [8, 1], FP)
    ones128 = din("ones128", [128, 1], FP)
    onesrow = din("onesrow", [1, 128], FP)
    onesS = din("onesS", [1, S], FP)

    yout = nc.dram_tensor("yout", [128, S], FP, kind="ExternalOutput").ap()

    rg = [list(range(N_CORES))]

    with tile.TileContext(nc) as tc:
        with (
            tc.tile_pool(name="const", bufs=1) as cp,
            tc.tile_pool(name="sb", bufs=3) as sb,
            tc.tile_pool(name="pgates", bufs=2, space="PSUM") as pgates,
            tc.tile_pool(name="pmisc", bufs=2, space="PSUM") as pmisc,
            tc.tile_pool(name="pbig", bufs=2, space="PSUM") as pbig,
            tc.tile_pool(name="dram", bufs=8, space="DRAM") as dram,
        ):
            # ---- load constants ----
            whh_s = cp.tile([128, 8, 4, 128], BF)
            nc.sync.dma_start(whh_s[:], whh[:])
            wih_s = cp.tile([128, 8, 4, 128], BF)
            nc.sync.dma_start(wih_s[:], wih[:])
            xT_s = cp.tile([128, 8, S], BF)
            nc.sync.dma_start(xT_s[:], xT[:])
            ebrow_s = cp.tile([1, 4, 128], FP)
            nc.sync.dma_start(ebrow_s[:], ebrow[:])
            atteh_s = cp.tile([128, 8, 128], BF)
            nc.sync.dma_start(atteh_s[:], atteh[:])
            attch_s = cp.tile([128, 8, 128], BF)
            nc.sync.dma_start(attch_s[:], attch[:])
            bc_s = cp.tile([128, 1], FP)
            nc.sync.dma_start(bc_s[:], bc[:])
            v2s_s = cp.tile([128, 1], BF)
            nc.sync.dma_start(v2s_s[:], v2s[:])
            dw_s = cp.tile([128, 16, 4, 128], BF)
            nc.sync.dma_start(dw_s[:], dw[:])
            dbrow_s = cp.tile([128, 4], FP)
            nc.sync.dma_start(dbrow_s[:], dbrow[:])
            eyef_s = cp.tile([128, 128], FP)
            nc.sync.dma_start(eyef_s[:], eye_f[:])
            eyeb_s = cp.tile([128, 128], BF)
            nc.sync.dma_start(eyeb_s[:], eye_b[:])
            ones8_s = cp.tile([8, 1], FP)
            nc.sync.dma_start(ones8_s[:], ones8[:])
            ones128_s = cp.tile([128, 1], FP)
            nc.sync.dma_start(ones128_s[:], ones128[:])
            onesrow_s = cp.tile([1, 128], FP)
            nc.sync.dma_start(onesrow_s[:], onesrow[:])
            onesS_s = cp.tile([1, S], FP)
            nc.sync.dma_start(onesS_s[:], onesS[:])

            # persistent buffers
            xpart = cp.tile([128, 4, S], FP)  # Wih@x + bias, T-layout
            e_outT = cp.tile([128, 8, S], BF)  # full h history, T-layout
            ehT = cp.tile([128, S], BF)
            e_outS = cp.tile([128, st, 8, 128], BF)  # e_out, S-partition layout
            zcol = cp.tile([128, 8], BF, tag="zeros8")
            nc.vector.memset(zcol[:], 0.0)
            zero1 = cp.tile([128, 1], FP, tag="zeros1")
            nc.vector.memset(zero1[:], 0.0)
            out_sb = cp.tile([128, S], FP)

            # ---- phase 0: x_part[g] = Wih_g @ x + b_g  (T-layout [128,4,S]) ----
            for gt in range(4):
                pxp = pbig.tile([128, S], FP, tag="pbig")
                for kc in range(8):
                    nc.tensor.matmul(
                        pxp[:],
                        wih_s[:, kc, gt, :],
                        xT_s[:, kc, :],
                        start=(kc == 0),
                        stop=False,
                    )
                nc.tensor.matmul(
                    pxp[:],
                    ebrow_s[:, gt, :],
                    onesS_s[:],
                    start=False,
                    stop=True,
                )
                nc.vector.tensor_copy(xpart[:, gt, :], pxp[:])

            # ---- encoder ----
            c_state = zero1
            for t in range(S):
                h_rhs = zcol if t == 0 else e_outT[:, :, t - 1 : t]
                pg = pgates.tile([128, 4], FP, tag="pg")
                for gt in range(4):
                    for kc in range(8):
                        if t == 0:
                            rhs = zcol[:, kc : kc + 1]
                        else:
                            rhs = e_outT[:, kc, t - 1 : t]
                        nc.tensor.matmul(
                            pg[:, gt : gt + 1],
                            whh_s[:, kc, gt, :],
                            rhs,
                            start=(kc == 0),
                            stop=(kc == 7),
                        )
                i_s = sb.tile([128, 1], FP, tag="i_s")
                nc.scalar.activation(
                    i_s[:], pg[:, 0:1], AF.Sigmoid, bias=xpart[:, 0, t : t + 1]
                )
                f_s = sb.tile([128, 1], FP, tag="f_s")
                nc.scalar.activation(
                    f_s[:], pg[:, 1:2], AF.Sigmoid, bias=xpart[:, 1, t : t + 1]
                )
                o_s = sb.tile([128, 1], FP, tag="o_s")
                nc.scalar.activation(
                    o_s[:], pg[:, 2:3], AF.Sigmoid, bias=xpart[:, 2, t : t + 1]
                )
                g_t = sb.tile([128, 1], FP, tag="g_t")
                nc.scalar.activation(
                    g_t[:], pg[:, 3:4], AF.Tanh, bias=xpart[:, 3, t : t + 1]
                )
                t1 = sb.tile([128, 1], FP, tag="t1")
                nc.vector.tensor_tensor(t1[:], f_s[:], c_state[:], OP.mult)
                t2 = sb.tile([128, 1], FP, tag="t2")
                nc.vector.tensor_tensor(t2[:], i_s[:], g_t[:], OP.mult)
                c_state = sb.tile([128, 1], FP, tag="c_st")
                nc.vector.tensor_tensor(c_state[:], t1[:], t2[:], OP.add)
                tc2 = sb.tile([128, 1], FP, tag="tc2")
                nc.scalar.activation(tc2[:], c_state[:], AF.Tanh)
                h2 = sb.tile([128, 1], FP, tag="h2")
                nc.vector.tensor_tensor(h2[:], o_s[:], tc2[:], OP.mult)

                # exchange: [128,1] -> [1,128] -> AG -> [8,128] -> [128,8]
                pt1 = pmisc.tile([1, 128], FP, tag="pm")
                nc.tensor.transpose(pt1[:], h2[:], eyef_s[:])
                h2row = sb.tile([1, 128], FP, tag="h2row")
                nc.vector.tensor_copy(h2row[:], pt1[:])
                b_in = dram.tile([1, 128], FP, tag="b_in")
                b_out = dram.tile([8, 128], FP, tag="b_out")
                nc.sync.dma_start(b_in[:], h2row[:])
                nc.gpsimd.collective_compute(
                    "AllGather", OP.bypass, replica_groups=rg,
                    ins=[b_in[:]], outs=[b_out[:]],
                )
                gath = sb.tile([8, 128], FP, tag="gath")
                nc.sync.dma_start(gath[:], b_out[:])
                pt2 = pbig.tile([128, 8], FP, tag="pbig")
                nc.tensor.transpose(pt2[:], gath[:], eyef_s[0:8, 0:8])
                nc.vector.tensor_copy(e_outT[:, :, t], pt2[:])

            # ---- phase: ehT = att_eh_W_c @ e_out^T (raw, bias later) ----
            peh = pbig.tile([128, S], FP, tag="pbig")
            for kc in range(8):
                nc.tensor.matmul(
                    peh[:],
                    atteh_s[:, kc, :],
                    e_outT[:, kc, :],
                    start=(kc == 0),
                    stop=(kc == 7),
                )
            nc.vector.tensor_copy(ehT[:], peh[:])

            # ---- phase: e_outS via 32 PE transposes ----
            for hj in range(8):
                for sc in range(st):
                    ptr = pbig.tile([128, 128], BF, tag="pbig")
                    nc.tensor.transpose(
                        ptr[:], e_outT[:, hj, 128 * sc : 128 * (sc + 1)], eyeb_s[:]
                    )
                    nc.vector.tensor_copy(e_outS[:, sc, hj, :], ptr[:])

            # ---- decoder ----
            c_dec = zero1
            ctxT = zcol
            for t in range(S):
                # gates_ctx half early (PE fills idle during attention)
                pgd = pgates.tile([128, 4], FP, tag="pg")
                for gt in range(4):
                    for kc in range(8):
                        nc.tensor.matmul(
                            pgd[:, gt : gt + 1],
                            dw_s[:, 8 + kc, gt, :],
                            ctxT[:, kc : kc + 1],
                            start=(kc == 0),
                            stop=False,
                        )
                # q = att_ch_W_c @ ctx ; qB = q + bc
                pq = pmisc.tile([128, 1], FP, tag="pm")
                for kc in range(8):
                    nc.tensor.matmul(
                        pq[:],
                        attch_s[:, kc, :],
                        ctxT[:, kc : kc + 1],
                        start=(kc == 0),
                        stop=(kc == 7),
                    )
                qB = sb.tile([128, 1], FP, tag="qB")
                nc.vector.tensor_tensor(qB[:], pq[:], bc_s[:], OP.add)
                # mT = tanh(ehT + qB)
                mT = sb.tile([128, S], BF, tag="mT")
                nc.scalar.activation(mT[:], ehT[:], AF.Tanh, bias=qB[:])
                # logits partial [1, S]
                plog = pbig.tile([1, S], FP, tag="pbig")
                nc.tensor.matmul(plog[:], v2s_s[:], mT[:], start=True, stop=True)
                logrow = sb.tile([1, S], FP, tag="logrow")
                nc.vector.tensor_copy(logrow[:], plog[:])
                b2_in = dram.tile([1, S], FP, tag="b2_in")
                b2_out = dram.tile([8, S], FP, tag="b2_out")
                nc.sync.dma_start(b2_in[:], logrow[:])
                nc.gpsimd.collective_compute(
                    "AllGather", OP.bypass, replica_groups=rg,
                    ins=[b2_in[:]], outs=[b2_out[:]],
                )
                gath2 = sb.tile([8, S], FP, tag="gath2")
                nc.sync.dma_start(gath2[:], b2_out[:])
                # logitsT [128, st] = rank-sums, transposed
                plt = pgates.tile([128, st], FP, tag="pg")
                for sc in range(st):
                    nc.tensor.matmul(
                        plt[:, sc : sc + 1],
                        gath2[0:8, 128 * sc : 128 * (sc + 1)],
                        ones8_s[:],
                        start=True,
                        stop=True,
                    )
                sT = sb.tile([128, st], BF, tag="sT")
                acc = sb.tile([128, 1], FP, tag="acc")
                nc.scalar.activation(sT[:], plt[:], AF.Exp, accum_out=acc[:])
                # total = sum over partitions; recip; broadcast to [128,1]
                psum1 = pmisc.tile([1, 1], FP, tag="pm")
                nc.tensor.matmul(psum1[:], acc[:], ones128_s[:], start=True, stop=True)
                recip = sb.tile([1, 1], FP, tag="recip")
                nc.vector.reciprocal(recip[:], psum1[:])
                prb = pmisc.tile([128, 1], FP, tag="pm")
                nc.tensor.matmul(prb[:], onesrow_s[:], recip[:], start=True, stop=True)
                recipT = sb.tile([128, 1], FP, tag="recipT")
                nc.vector.tensor_copy(recipT[:], prb[:])
                # z (full, replicated): [128, 8]
                pz = pbig.tile([128, 8], FP, tag="pbig")
                for hj in range(8):
                    for sc in range(st):
                        nc.tensor.matmul(
                            pz[:, hj : hj + 1],
                            e_outS[:, sc, hj, :],
                            sT[:, sc : sc + 1],
                            start=(sc == 0),
                            stop=(sc == st - 1),
                        )
                zT = sb.tile([128, 8], BF, tag="zT")
                nc.vector.tensor_scalar_mul(zT[:], pz[:], recipT[:, 0:1])
                # gates_z half
                for gt in range(4):
                    for kc in range(8):
                        nc.tensor.matmul(
                            pgd[:, gt : gt + 1],
                            dw_s[:, kc, gt, :],
                            zT[:, kc : kc + 1],
                            start=False,
                            stop=(kc == 7),
                        )
                # pointwise
                i_s = sb.tile([128, 1], FP, tag="i_s")
                nc.scalar.activation(
                    i_s[:], pgd[:, 0:1], AF.Sigmoid, bias=dbrow_s[:, 0:1]
                )
                f_s = sb.tile([128, 1], FP, tag="f_s")
                nc.scalar.activation(
                    f_s[:], pgd[:, 1:2], AF.Sigmoid, bias=dbrow_s[:, 1:2]
                )
                o_s = sb.tile([128, 1], FP, tag="o_s")
                nc.scalar.activation(
                    o_s[:], pgd[:, 2:3], AF.Sigmoid, bias=dbrow_s[:, 2:3]
                )
                g_t = sb.tile([128, 1], FP, tag="g_t")
                nc.scalar.activation(
                    g_t[:], pgd[:, 3:4], AF.Tanh, bias=dbrow_s[:, 3:4]
                )
                t1 = sb.tile([128, 1], FP, tag="t1")
                nc.vector.tensor_tensor(t1[:], f_s[:], c_dec[:], OP.mult)
                t2 = sb.tile([128, 1], FP, tag="t2")
                nc.vector.tensor_tensor(t2[:], i_s[:], g_t[:], OP.mult)
                c_dec = sb.tile([128, 1], FP, tag="c_st")
                nc.vector.tensor_tensor(c_dec[:], t1[:], t2[:], OP.add)
                tc2 = sb.tile([128, 1], FP, tag="tc2")
                nc.scalar.activation(tc2[:], c_dec[:], AF.Tanh)
                nc.vector.tensor_tensor(
                    out_sb[:, t : t + 1], o_s[:], tc2[:], OP.mult
                )

                if t < S - 1:
                    # exchange decoder h -> ctxT
                    pt1 = pmisc.tile([1, 128], FP, tag="pm")
                    nc.tensor.transpose(pt1[:], out_sb[:, t : t + 1], eyef_s[:])
                    h2row = sb.tile([1, 128], FP, tag="h2row")
                    nc.vector.tensor_copy(h2row[:], pt1[:])
                    b_in = dram.tile([1, 128], FP, tag="b_in")
                    b_out = dram.tile([8, 128], FP, tag="b_out")
                    nc.sync.dma_start(b_in[:], h2row[:])
                    nc.gpsimd.collective_compute(
                        "AllGather", OP.bypass, replica_groups=rg,
                        ins=[b_in[:]], outs=[b_out[:]],
                    )
                    gath = sb.tile([8, 128], FP, tag="gath")
                    nc.sync.dma_start(gath[:], b_out[:])
                    pt2 = pbig.tile([128, 8], FP, tag="pbig")
                    nc.tensor.transpose(pt2[:], gath[:], eyef_s[0:8, 0:8])
                    ctxT = sb.tile([128, 8], BF, tag="ctxT")
                    nc.vector.tensor_copy(ctxT[:], pt2[:])

            nc.sync.dma_start(yout[:], out_sb[:])

    nc.compile()
    return nc


def prep_inputs(S, x, e_Wih, e_Whh, e_bih, e_bhh, att_eh_W, att_eh_b, att_ch_W,
                att_ch_b, v2s_W, v2s_b, att_bias, d_Wih, d_Whh, d_bih, d_bhh):
    """Per-core input maps (host-side slicing/transposition/casting only)."""
    x = np.asarray(x, np.float32)
    xe = x[:, 0, :]  # (S, I)
    eb = np.asarray(e_bih, np.float32) + np.asarray(e_bhh, np.float32)
    db = np.asarray(d_bih, np.float32) + np.asarray(d_bhh, np.float32)
    bsum = (
        np.asarray(att_eh_b, np.float32)
        + np.asarray(att_bias, np.float32)
        + np.asarray(att_ch_b, np.float32)
    )
    e_Wih = np.asarray(e_Wih, np.float32)
    e_Whh = np.asarray(e_Whh, np.float32)
    att_eh_W = np.asarray(att_eh_W, np.float32)
    att_ch_W = np.asarray(att_ch_W, np.float32)
    v2s_W = np.asarray(v2s_W, np.float32)
    d_Wih = np.asarray(d_Wih, np.float32)
    d_Whh = np.asarray(d_Whh, np.float32)

    eye128 = np.eye(128, dtype=np.float32)
    in_maps = []
    for c in range(N_CORES):
        hs = slice(128 * c, 128 * (c + 1))
        whh_t = np.empty((128, 8, 4, 128), np.float32)
        wih_t = np.empty((128, 8, 4, 128), np.float32)
        dw_t = np.empty((128, 16, 4, 128), np.float32)
        ebrow = np.empty((1, 4, 128), np.float32)
        dbrow = np.empty((128, 4), np.float32)
        for gt in range(4):
            rows = slice(GMAP[gt] * H + 128 * c, GMAP[gt] * H + 128 * (c + 1))
            for kc in range(8):
                kcs = slice(128 * kc, 128 * (kc + 1))
                whh_t[:, kc, gt, :] = e_Whh[rows, kcs].T
                wih_t[:, kc, gt, :] = e_Wih[rows, kcs].T
                dw_t[:, kc, gt, :] = d_Wih[rows, kcs].T
                dw_t[:, 8 + kc, gt, :] = d_Whh[rows, kcs].T
            ebrow[0, gt, :] = eb[rows]
            dbrow[:, gt] = db[rows]
        atteh_t = np.empty((128, 8, 128), np.float32)
        attch_t = np.empty((128, 8, 128), np.float32)
        for kc in range(8):
            kcs = slice(128 * kc, 128 * (kc + 1))
            atteh_t[:, kc, :] = att_eh_W[hs, kcs].T
            attch_t[:, kc, :] = att_ch_W[hs, kcs].T
        xT_t = np.empty((128, 8, S), np.float32)
        for kc in range(8):
            xT_t[:, kc, :] = xe[:S, 128 * kc : 128 * (kc + 1)].T
        in_maps.append(
            {
                "whh": whh_t.astype(NP_BF16),
                "wih": wih_t.astype(NP_BF16),
                "xT": xT_t.astype(NP_BF16),
                "ebrow": ebrow,
                "atteh": atteh_t.astype(NP_BF16),
                "attch": attch_t.astype(NP_BF16),
                "bc": bsum[hs].reshape(128, 1),
                "v2s": v2s_W[0, hs].reshape(128, 1).astype(NP_BF16),
                "dw": dw_t.astype(NP_BF16),
                "dbrow": dbrow,
                "eye_f": eye128,
                "eye_b": eye128.astype(NP_BF16),
                "ones8": np.ones((8, 1), np.float32),
                "ones128": np.ones((128, 1), np.float32),
                "onesrow": np.ones((1, 128), np.float32),
                "onesS": np.ones((1, S), np.float32),
            }
        )
    return in_maps


_NC_CACHE = {}


def _get_nc(S):
    if S not in _NC_CACHE:
        _NC_CACHE[S] = build_kernel(S)
    return _NC_CACHE[S]


def kernel(**inputs) -> np.ndarray:
    global LAST_EXEC_NS
    S = inputs["x"].shape[0]
    nc = _get_nc(S)
    in_maps = prep_inputs(S, **inputs)
    trace = bool(os.environ.get("BASS_TRACE"))
    if trace:
        from trn_agent_boot.trn_boot import _ntff_profile_via_ctypes

        sys.modules["antenv.axon_hooks"].set_axon_ntff_profile_hook(
            _ntff_profile_via_ctypes("/opt/axon/libaxon_pjrt.so")
        )
    res = bass_utils.run_bass_kernel_spmd(
        nc, in_maps, core_ids=list(range(N_CORES)), trace=trace
    )
    LAST_EXEC_NS = res.exec_time_ns
    out = np.empty((S, 1, H), np.float32)
    for c in range(N_CORES):
        out[:, 0, 128 * c : 128 * (c + 1)] = res.results[c]["yout"].T
    return out
